# revision 4
# baseline (speedup 1.0000x reference)
"""AttentionVisual Bass/TRN2 kernel.

reference:
    v_proj = vf @ Wv + bv            # [B,R,A]
    h_proj = hidden @ Wh + bh        # [B,A]
    joint  = relu(v_proj + h_proj)   # [B,R,A]
    scores = joint @ Wa + ba         # [B,R]
    att    = softmax(scores, R)      # ba shifts all scores equally -> softmax invariant
    out    = einsum('br,brd->bd', att, vf)

Sharding: batch (512) data-parallel over 8 cores -> 64 batches/core.

Per-core dataflow (batches processed in pairs so matmul free dim >= 256,
which is what lets float32r stream at 1 cycle/row on the PE):
  1. DMA vf[b] in natural layout [R=196 rows -> partitions, Dv=1024 free].
  2. PE-transpose (fp32, exact) each [r,128] block -> vfT [Dv on partitions,
     2*196 free] staged through PSUM, copied to SBUF on ACT/DVE.
  3. mm1: Wv stationary [Dv,A], vfT moving (float32r) -> v_projT [A,392] PSUM.
  4. joint: DVE tensor_scalar (add h_projT col, relu) -> SBUF.
  5. mm2: Wa stationary [A,1], joint moving (float32r) -> scoresT [1,392].
  6. ACT exp, DVE row-softmax (no max-subtract: scores are O(1) by
     construction), PE row->col transpose of att (tiny matmuls vs ones).
  7. pooling: att col stationary [r,1], vf natural moving (float32r)
     -> out [1,1024] PSUM, copied out and DMA'd per batch row.
"""

import os
import sys

import numpy as np

sys.path.insert(0, "/opt/trn_rl_repo")

VIS_DIM, HID_DIM, ATT_DIM = 1024, 512, 256
B, R = 512, 196
N_CORES = 8
BC = B // N_CORES          # 64 batches per core
NPAIR = BC // 2            # 32
RC0, RC1 = 128, 68         # R = 128 + 68
R2 = 2 * R                 # free dim of paired tiles = 392


def build_nc():
    import concourse.mybir as mybir
    import concourse.tile as tile
    from concourse.bacc import Bacc

    f32 = mybir.dt.float32
    f32r = mybir.dt.float32r
    AF = mybir.ActivationFunctionType
    OP = mybir.AluOpType

    nc = Bacc()

    vf_d = nc.dram_tensor("visual_features", [BC, R, VIS_DIM], f32, kind="ExternalInput")
    hid_d = nc.dram_tensor("hiddenT", [HID_DIM, BC], f32, kind="ExternalInput")
    wv_d = nc.dram_tensor("Wv", [VIS_DIM, ATT_DIM], f32, kind="ExternalInput")
    wh_d = nc.dram_tensor("Wh", [HID_DIM, ATT_DIM], f32, kind="ExternalInput")
    bb_d = nc.dram_tensor("bhbv", [ATT_DIM, 1], f32, kind="ExternalInput")
    wa_d = nc.dram_tensor("Wa", [ATT_DIM, 1], f32, kind="ExternalInput")
    ident_d = nc.dram_tensor("ident", [128, 128], f32, kind="ExternalInput")
    out_d = nc.dram_tensor("att_out", [BC, VIS_DIM], f32, kind="ExternalOutput")

    with tile.TileContext(nc) as tc:
        with (
            tc.tile_pool(name="const", bufs=1) as cpool,
            tc.tile_pool(name="vfn", bufs=12) as vfn_pool,
            tc.tile_pool(name="vfT", bufs=16) as vfT_pool,
            tc.tile_pool(name="joint", bufs=4) as joint_pool,
            tc.tile_pool(name="small", bufs=4) as small_pool,
            tc.tile_pool(name="ps_t", bufs=2, space="PSUM") as ps_t_pool,
            tc.tile_pool(name="ps_vp", bufs=2, space="PSUM") as ps_vp_pool,
            tc.tile_pool(name="ps_sc", bufs=1, space="PSUM") as ps_sc_pool,
            tc.tile_pool(name="ps_at", bufs=1, space="PSUM") as ps_at_pool,
            tc.tile_pool(name="ps_po", bufs=2, space="PSUM") as ps_po_pool,
        ):
            # ---- constants ----
            ident = cpool.tile([128, 128], f32, tag="ident")
            nc.sync.dma_start(ident[:, :], ident_d[:, :])
            identr = cpool.tile([128, 128], f32r, tag="identr")
            nc.gpsimd.dma_start(identr[:, :], ident_d[:, :])

            wv_sb = []
            for dc in range(8):
                t = cpool.tile([128, ATT_DIM], f32r, tag=f"wv{dc}")
                nc.gpsimd.dma_start(t[:, :], wv_d[dc * 128:(dc + 1) * 128, :])
                wv_sb.append(t)

            wh_sb = []
            for kd in range(4):
                t = cpool.tile([128, ATT_DIM], f32, tag=f"wh{kd}")
                nc.sync.dma_start(t[:, :], wh_d[kd * 128:(kd + 1) * 128, :])
                wh_sb.append(t)

            bias_sb = []   # bv + bh combined on host, [128,1] per A-half
            wa_sb = []
            for ah in range(2):
                bt = cpool.tile([128, 1], f32, tag=f"bias{ah}")
                wat = cpool.tile([128, 1], f32r, tag=f"wa{ah}")
                nc.sync.dma_start(bt[:, :], bb_d[ah * 128:(ah + 1) * 128, :])
                nc.gpsimd.dma_start(wat[:, :], wa_d[ah * 128:(ah + 1) * 128, :])
                bias_sb.append(bt)
                wa_sb.append(wat)

            # ---- h_projT = (hidden @ Wh).T + (bh + bv)  -> [A on partitions, 64] ----
            hT_sb = []
            for kd in range(4):
                t = cpool.tile([128, BC], f32, tag=f"hT{kd}")
                nc.sync.dma_start(t[:, :], hid_d[kd * 128:(kd + 1) * 128, :])
                hT_sb.append(t)

            hp_sb = []
            for ah in range(2):
                ps = ps_vp_pool.tile([128, BC], f32, tag="ps_vp")
                for kd in range(4):
                    nc.tensor.matmul(
                        ps[:, :],
                        wh_sb[kd][:, ah * 128:(ah + 1) * 128],
                        hT_sb[kd][:, :],
                        start=(kd == 0), stop=(kd == 3),
                    )
                t = cpool.tile([128, BC], f32, tag=f"hp{ah}")
                nc.vector.tensor_scalar(t[:, :], ps[:, :], bias_sb[ah][:, :], None, OP.add)
                hp_sb.append(t)

            # ---- main loop over batch pairs ----
            for pair in range(NPAIR):
                bs = [2 * pair, 2 * pair + 1]

                # natural-layout loads: [r, 1024], r on partitions
                vfn = []
                for q in range(2):
                    ta = vfn_pool.tile([128, VIS_DIM], f32r, tag="vfn")
                    nc.gpsimd.dma_start(ta[:, :], vf_d[bs[q], 0:RC0, :])
                    tb = vfn_pool.tile([128, VIS_DIM], f32r, tag="vfn")
                    nc.gpsimd.dma_start(tb[0:RC1, :], vf_d[bs[q], RC0:R, :])
                    vfn.append((ta, tb))

                # transpose -> vfT[dc] = [128 (Dv chunk), 392 (= 2*(128+68))]
                vfT = []
                for dc in range(8):
                    ps = ps_t_pool.tile([128, R2], f32r, tag="ps_t")
                    for q in range(2):
                        nc.tensor.transpose(
                            ps[:, q * R:q * R + RC0],
                            vfn[q][0][:, dc * 128:(dc + 1) * 128],
                            identr[:, :])
                        nc.tensor.transpose(
                            ps[:, q * R + RC0:(q + 1) * R],
                            vfn[q][1][0:RC1, dc * 128:(dc + 1) * 128],
                            identr[0:RC1, 0:RC1])
                    t = vfT_pool.tile([128, R2], f32r, tag="vfT")
                    if dc % 2 == 0:
                        nc.scalar.copy(t[:, :], ps[:, :])
                    else:
                        nc.vector.tensor_copy(t[:, :], ps[:, :])
                    vfT.append(t)

                # mm1 + joint
                joint = []
                for ah in range(2):
                    ps = ps_vp_pool.tile([128, R2], f32, tag="ps_vp")
                    for dc in range(8):
                        nc.tensor.matmul(
                            ps[:, :],
                            wv_sb[dc][:, ah * 128:(ah + 1) * 128],
                            vfT[dc][:, :],
                            start=(dc == 0), stop=(dc == 7),
                        )
                    jt = joint_pool.tile([128, R2], f32r, tag="joint")
                    for q in range(2):
                        nc.vector.tensor_scalar(
                            jt[:, q * R:(q + 1) * R],
                            ps[:, q * R:(q + 1) * R],
                            hp_sb[ah][:, bs[q]:bs[q] + 1],
                            0.0, OP.add, OP.max)
                    joint.append(jt)

                # mm2 -> scoresT [1, 392]
                sc_ps = ps_sc_pool.tile([1, R2], f32, tag="ps_sc")
                for ah in range(2):
                    nc.tensor.matmul(
                        sc_ps[:, :],
                        wa_sb[ah][:, :],
                        joint[ah][:, :],
                        start=(ah == 0), stop=(ah == 1),
                    )

                # softmax over R per batch (no max-subtract; scores are O(1))
                exp_row = small_pool.tile([1, R2], f32, tag="exp")
                nc.scalar.activation(exp_row[:, :], sc_ps[:, :], AF.Exp)
                att_row = small_pool.tile([1, R2], f32, tag="att")
                for q in range(2):
                    z = small_pool.tile([1, 1], f32, tag="z")
                    nc.vector.tensor_reduce(
                        z[:, :], exp_row[:, q * R:(q + 1) * R],
                        mybir.AxisListType.X, OP.add)
                    rz = small_pool.tile([1, 1], f32, tag="rz")
                    nc.vector.reciprocal(rz[:, :], z[:, :])
                    nc.vector.tensor_scalar(
                        att_row[:, q * R:(q + 1) * R],
                        exp_row[:, q * R:(q + 1) * R],
                        rz[:, :], None, OP.mult)

                # att row -> col  [r, 1] per (q, rc) via tiny matmuls vs ones col
                atc_ps = ps_at_pool.tile([128, 4], f32, tag="ps_at")
                for q in range(2):
                    nc.tensor.matmul(
                        atc_ps[0:RC0, 2 * q:2 * q + 1],
                        att_row[0:1, q * R:q * R + RC0],
                        ident[0:1, 0:1],
                        start=True, stop=True)
                    nc.tensor.matmul(
                        atc_ps[0:RC1, 2 * q + 1:2 * q + 2],
                        att_row[0:1, q * R + RC0:(q + 1) * R],
                        ident[0:1, 0:1],
                        start=True, stop=True)
                atc = small_pool.tile([128, 4], f32r, tag="atc")
                nc.vector.tensor_copy(atc[:, :], atc_ps[:, :])

                # pooling: out[b] = att . vf[b]   [1, 1024]
                for q in range(2):
                    orow = small_pool.tile([1, VIS_DIM], f32, tag="orow")
                    for nch in range(2):
                        po = ps_po_pool.tile([1, 512], f32, tag="ps_po")
                        nc.tensor.matmul(
                            po[:, :],
                            atc[0:RC0, 2 * q:2 * q + 1],
                            vfn[q][0][:, nch * 512:(nch + 1) * 512],
                            start=True, stop=False)
                        nc.tensor.matmul(
                            po[:, :],
                            atc[0:RC1, 2 * q + 1:2 * q + 2],
                            vfn[q][1][0:RC1, nch * 512:(nch + 1) * 512],
                            start=False, stop=True)
                        if nch == 0:
                            nc.scalar.copy(orow[:, 0:512], po[:, :])
                        else:
                            nc.vector.tensor_copy(orow[:, 512:1024], po[:, :])
                    nc.sync.dma_start(out_d[bs[q]:bs[q] + 1, :], orow[:, :])

    nc.compile()
    return nc


_NC_CACHE = {}


def kernel(**inputs):
    vf = np.ascontiguousarray(inputs["visual_features"], dtype=np.float32)
    hid = np.ascontiguousarray(inputs["hidden_state"], dtype=np.float32)
    Wv = np.ascontiguousarray(inputs["Wv"], dtype=np.float32)
    bv = np.asarray(inputs["bv"], dtype=np.float32).reshape(ATT_DIM, 1)
    Wh = np.ascontiguousarray(inputs["Wh"], dtype=np.float32)
    bh = np.asarray(inputs["bh"], dtype=np.float32).reshape(ATT_DIM, 1)
    bhbv = np.ascontiguousarray(bv + bh)
    Wa = np.ascontiguousarray(inputs["Wa"], dtype=np.float32).reshape(ATT_DIM, 1)
    # ba shifts every score equally; softmax is shift-invariant, so it cannot
    # affect the output and is deliberately unused.

    from concourse.bass_utils import run_bass_kernel_spmd

    if "nc" not in _NC_CACHE:
        _NC_CACHE["nc"] = build_nc()
    nc = _NC_CACHE["nc"]

    ident = np.eye(128, dtype=np.float32)
    in_maps = []
    for c in range(N_CORES):
        sl = slice(c * BC, (c + 1) * BC)
        in_maps.append({
            "visual_features": np.ascontiguousarray(vf[sl]),
            "hiddenT": np.ascontiguousarray(hid[sl].T),
            "Wv": Wv, "Wh": Wh, "bhbv": bhbv, "Wa": Wa,
            "ident": ident,
        })

    res = run_bass_kernel_spmd(nc, in_maps, core_ids=list(range(N_CORES)),
                               trace=os.environ.get("KBENCH_TRACE", "") == "1")
    out = np.concatenate([r["att_out"] for r in res.results], axis=0)
    _NC_CACHE["last_result"] = res
    return out
